# revision 18
# baseline (speedup 1.0000x reference)
"""EquivariantGraphConv on 8 Trainium2 NeuronCores.

Strategy (edge-parallel, dst-ownership sharding):
  - Nodes are partitioned into 8 contiguous ranges of 2560 (core 7 padded).
  - Each core owns 20 node blocks of 128; every edge is routed to the core
    (and 128-node block) that owns its dst, so segment sums need no
    cross-core reduction at all.
  - Host pads each (core, block) edge list to a fixed capacity so the
    compiled program is identical on all cores (SPMD).
  - On device, per edge block: gather src features via dma_gather from a
    precomputed table P = node_feat @ eW1[:128] (built on device), run the
    edge MLP feature-major in bf16, build one-hot dst matrices with
    iota/is_equal and use PE matmuls for the dst gathers (Q = nf @ eW1[128:256],
    coords[dst]) and for the segment-sum scatter (PSUM accumulation).
  - Node MLP + residuals run per block; outputs are returned as per-core
    shards and concatenated on the host.
"""

import sys

sys.path.insert(0, "/opt/trn_rl_repo")

import numpy as np
import ml_dtypes

import concourse.bass as bass
import concourse.bacc as bacc
import concourse.mybir as mybir
import concourse.tile as tile
from concourse.bass_utils import run_bass_kernel_spmd

BF16 = ml_dtypes.bfloat16
F32 = np.float32

# Problem shapes (hardcoded per spec)
NN = 20000          # nodes
E = 640000          # edges
D = 128             # node feat dim == HID
EA = 64             # edge attr dim
HID = 128
EPS = 1e-8

NCORES = 8
NBLK = 20           # node blocks per core
NPC = NBLK * 128    # nodes per core (2560)
GBLK = 160          # padded global block count (157 real)
NPAD = 157 * 128    # padded node count for tables (20096)
EPB = 4608          # edge capacity per (core, block); Poisson(4096)+8sigma
TPB = EPB // 128    # 36 tiles / block
CPB = EPB // 512    # 9 chunks / block
CH = 512
EPC = NBLK * EPB    # 92160 edges per core

_PROG_CACHE = {}

import os
_DBG_PHASES = os.environ.get("K_PHASES", "ABC")  # subset of "ABC" for bisection
_DBG_NBLK = int(os.environ.get("K_NBLK", str(NBLK)))
_DBG_CLVL = int(os.environ.get("K_CLVL", "7"))


def _to_gather_idx_layout(idx_i16: np.ndarray) -> np.ndarray:
    """dma_gather index layout: logical i -> partition i%16, col i//16,
    replicated across the 8 groups of 16 partitions."""
    s = idx_i16.reshape(-1, 16).T  # [16, n/16]
    return np.ascontiguousarray(np.tile(s, (8, 1)))  # [128, n/16]


def _prep(node_feat, edge_index, edge_attr, coords,
          eW1, eb1, eW2, eb2, nW1, nb1, nW2, nb2, cW1, cb1, cW2, cb2):
    node_feat = np.asarray(node_feat, dtype=F32)
    edge_index = np.asarray(edge_index)
    edge_attr = np.asarray(edge_attr, dtype=F32)
    coords = np.asarray(coords, dtype=F32)

    src = np.asarray(edge_index[0], dtype=np.int64)
    dst = np.asarray(edge_index[1], dtype=np.int64)

    g = dst >> 7                      # global 128-node block, 0..156
    order = np.argsort(g, kind="stable")
    counts = np.bincount(g, minlength=GBLK)
    if counts.max() > EPB:
        raise RuntimeError(f"block overflow: {counts.max()} > {EPB}")
    off = np.zeros(GBLK + 1, dtype=np.int64)
    off[1:] = np.cumsum(counts)

    es = order
    gs = g[es]
    pos = np.arange(E, dtype=np.int64) - off[gs]
    slot = gs * EPB + pos             # slot in [0, GBLK*EPB)

    TOT = GBLK * EPB
    src_pad = np.zeros(TOT, dtype=np.int16)
    dstb_pad = np.full(TOT, 192.0, dtype=F32)
    ea_pad = np.zeros((TOT, EA), dtype=BF16)
    src_pad[slot] = src[es].astype(np.int16)
    dstb_pad[slot] = (dst[es] & 127).astype(F32)
    ea_pad[slot] = edge_attr[es].astype(BF16)

    # node tables (shared across cores)
    nfT = np.zeros((D, NPAD), dtype=BF16)
    nfT[:, :NN] = node_feat.T.astype(BF16)
    coords_tab = np.zeros((NPAD, 64), dtype=F32)
    coords_tab[:NN, :3] = coords

    iota_col = np.arange(128, dtype=F32).reshape(128, 1).astype(BF16)
    iota_rep = np.broadcast_to(np.arange(128, dtype=F32).astype(BF16), (128, 128)).copy()
    ident_bf = np.eye(128, dtype=F32).astype(BF16)
    ident_f32 = np.eye(128, dtype=F32)

    # weights
    W1s = np.ascontiguousarray(eW1[0:128]).astype(BF16)         # [128,128]
    W1d = np.ascontiguousarray(eW1[128:256]).astype(BF16)       # [128,128]
    W1e = np.ascontiguousarray(eW1[256:320]).astype(BF16)       # [64,128]
    eW2b = np.asarray(eW2, dtype=F32).astype(BF16)
    cW1b = np.asarray(cW1, dtype=F32).astype(BF16)              # [128,64]
    cW2p8 = np.zeros((64, 8), dtype=BF16)
    cW2p8[:, 0] = np.asarray(cW2, dtype=F32)[:, 0].astype(BF16)
    nW1a = np.ascontiguousarray(np.asarray(nW1, dtype=F32)[0:128]).astype(BF16)
    nW1b = np.ascontiguousarray(np.asarray(nW1, dtype=F32)[128:256]).astype(BF16)
    nW2b = np.asarray(nW2, dtype=F32).astype(BF16)

    shared = {
        "nfT": nfT,
        "coords_tab": coords_tab,
        "iota_col": iota_col,
        "iota_rep": iota_rep,
        "ident_bf": ident_bf,
        "ident_f32": ident_f32,
        "W1s": W1s, "W1d": W1d, "W1e": W1e, "eW2": eW2b,
        "cW1": cW1b, "cW2p8": cW2p8,
        "nW1a": nW1a, "nW1b": nW1b, "nW2": nW2b,
        "eb1": np.asarray(eb1, F32).reshape(HID, 1),
        "eb2": np.asarray(eb2, F32).reshape(HID, 1),
        "cb1": np.asarray(cb1, F32).reshape(64, 1),
        "cb2": np.full((128, 1), np.asarray(cb2, F32).reshape(-1)[0], dtype=F32),
        "nb1": np.asarray(nb1, F32).reshape(HID, 1),
        "nb2": np.asarray(nb2, F32).reshape(D, 1),
    }

    in_maps = []
    for c in range(NCORES):
        lo = c * NBLK * EPB
        hi = lo + NBLK * EPB
        src_c = src_pad[lo:hi]
        dstb_c = dstb_pad[lo:hi]
        ea_c = ea_pad[lo:hi]

        # per-core owned-node slices (core 7 zero-padded)
        n0 = c * NPC
        n1 = min((c + 1) * NPC, NN)
        nfT_own = np.zeros((D, NPC), dtype=BF16)
        nfT_own[:, : n1 - n0] = node_feat[n0:n1].T.astype(BF16)
        nf_own = np.zeros((NPC, D), dtype=F32)
        nf_own[: n1 - n0] = node_feat[n0:n1]
        co_own = np.zeros((NPC, 3), dtype=F32)
        co_own[: n1 - n0] = coords[n0:n1]

        chi = co_own.astype(BF16)
        clo = (co_own - chi.astype(F32)).astype(BF16)
        chi8 = np.zeros((NPC, 8), dtype=BF16)
        clo8 = np.zeros((NPC, 8), dtype=BF16)
        chi8[:, 1:4] = chi
        clo8[:, 4:7] = clo

        m = {
            "src_idx": _to_gather_idx_layout(src_c),
            "dstb_fm": np.ascontiguousarray(dstb_c.astype(BF16).reshape(1, EPC)),
            "dstb_em": np.ascontiguousarray(dstb_c.astype(BF16).reshape(-1, 128).T),
            "eaT": np.ascontiguousarray(ea_c.T),
            "nfT_own": nfT_own,
            "nf_own_t": np.ascontiguousarray(
                nf_own.reshape(NBLK, 128, D).transpose(1, 0, 2)),
            "co_own_t": np.ascontiguousarray(
                co_own.reshape(NBLK, 128, 3).transpose(1, 0, 2)),
            "chi8_t": np.ascontiguousarray(
                chi8.reshape(NBLK, 128, 8).transpose(1, 0, 2)),
            "clo8_t": np.ascontiguousarray(
                clo8.reshape(NBLK, 128, 8).transpose(1, 0, 2)),
        }
        m.update(shared)
        in_maps.append(m)
    return in_maps


def _build_program():
    nc = bacc.Bacc("TRN2", target_bir_lowering=False, debug=False,
                   num_devices=NCORES)
    bf = mybir.dt.bfloat16
    f32 = mybir.dt.float32
    i16 = mybir.dt.int16

    def inp(name, shape, dt):
        return nc.dram_tensor(name, list(shape), dt, kind="ExternalInput").ap()

    src_idx = inp("src_idx", (128, EPC // 16), i16)
    dstb_fm = inp("dstb_fm", (1, EPC), bf)
    dstb_em = inp("dstb_em", (128, EPC // 128), bf)
    eaT = inp("eaT", (EA, EPC), bf)
    nfT = inp("nfT", (D, NPAD), bf)
    nfT_own = inp("nfT_own", (D, NPC), bf)
    nf_own_t = inp("nf_own_t", (128, NBLK, D), f32)
    co_own_t = inp("co_own_t", (128, NBLK, 3), f32)
    chi8_t = inp("chi8_t", (128, NBLK, 8), bf)
    clo8_t = inp("clo8_t", (128, NBLK, 8), bf)
    coords_tab = inp("coords_tab", (NPAD, 64), f32)
    iota_col_i = inp("iota_col", (128, 1), bf)
    iota_rep_i = inp("iota_rep", (128, 128), bf)
    ident_bf_i = inp("ident_bf", (128, 128), bf)
    ident_f32_i = inp("ident_f32", (128, 128), f32)
    W1s_i = inp("W1s", (128, 128), bf)
    W1d_i = inp("W1d", (128, 128), bf)
    W1e_i = inp("W1e", (64, 128), bf)
    eW2_i = inp("eW2", (128, 128), bf)
    cW1_i = inp("cW1", (128, 64), bf)
    cW2p8_i = inp("cW2p8", (64, 8), bf)
    nW1a_i = inp("nW1a", (128, 128), bf)
    nW1b_i = inp("nW1b", (128, 128), bf)
    nW2_i = inp("nW2", (128, 128), bf)
    eb1_i = inp("eb1", (HID, 1), f32)
    eb2_i = inp("eb2", (HID, 1), f32)
    cb1_i = inp("cb1", (64, 1), f32)
    cb2_i = inp("cb2", (128, 1), f32)
    nb1_i = inp("nb1", (HID, 1), f32)
    nb2_i = inp("nb2", (D, 1), f32)

    out_nodes = nc.dram_tensor("out_nodes", [NPC, D], f32,
                               kind="ExternalOutput").ap()
    out_coords = nc.dram_tensor("out_coords", [NPC, 3], f32,
                                kind="ExternalOutput").ap()
    P_tab = nc.dram_tensor("P_tab", [NPAD, D], bf, kind="Internal").ap()

    AX = mybir.AxisListType
    OP = mybir.AluOpType
    AF = mybir.ActivationFunctionType

    from contextlib import ExitStack
    with tile.TileContext(nc) as tc, ExitStack() as stk:
        const = stk.enter_context(tc.tile_pool(name="const", bufs=1))
        work = stk.enter_context(tc.tile_pool(name="work", bufs=2))
        wk3 = stk.enter_context(tc.tile_pool(name="wk3", bufs=3))
        psA = stk.enter_context(tc.tile_pool(name="psA", bufs=2, space="PSUM"))
        psB = stk.enter_context(tc.tile_pool(name="psB", bufs=2, space="PSUM"))
        psS = stk.enter_context(tc.tile_pool(name="psS", bufs=2, space="PSUM"))
        psT = stk.enter_context(tc.tile_pool(name="psT", bufs=1, space="PSUM"))
        psC = stk.enter_context(tc.tile_pool(name="psC", bufs=1, space="PSUM"))

        def cload(ap_in, shape, dtype):
            t = const.tile(shape, dtype, tag=f"c_{ap_in.tensor.name}")
            nc.sync.dma_start(t[:], ap_in[:])
            return t

        W1s_sb = cload(W1s_i, [128, 128], bf)
        W1d_sb = cload(W1d_i, [128, 128], bf)
        W1e_sb = cload(W1e_i, [64, 128], bf)
        eW2_sb = cload(eW2_i, [128, 128], bf)
        cW1_sb = cload(cW1_i, [128, 64], bf)
        cW2p8_sb = cload(cW2p8_i, [64, 8], bf)
        nW1a_sb = cload(nW1a_i, [128, 128], bf)
        nW1b_sb = cload(nW1b_i, [128, 128], bf)
        nW2_sb = cload(nW2_i, [128, 128], bf)
        eb1_sb = cload(eb1_i, [HID, 1], f32)
        eb2_sb = cload(eb2_i, [HID, 1], f32)
        cb1_sb = cload(cb1_i, [64, 1], f32)
        cb2_sb = cload(cb2_i, [128, 1], f32)
        nb1_sb = cload(nb1_i, [HID, 1], f32)
        nb2_sb = cload(nb2_i, [D, 1], f32)
        iota_col = cload(iota_col_i, [128, 1], bf)
        iota_rep = cload(iota_rep_i, [128, 128], bf)
        ident_bf = cload(ident_bf_i, [128, 128], bf)
        ident_f32 = cload(ident_f32_i, [128, 128], f32)
        src_idx_sb = cload(src_idx, [128, EPC // 16], i16)
        nfo_sb = cload(nfT_own, [D, NPC], bf)
        nfown_sb = cload(nf_own_t, [128, NBLK, D], f32)
        coown_sb = cload(co_own_t, [128, NBLK, 3], f32)
        chi8_sb = cload(chi8_t, [128, NBLK, 8], bf)
        clo8_sb = cload(clo8_t, [128, NBLK, 8], bf)
        dem_sb = cload(dstb_em, [128, EPC // 128], bf)

        # ---- Phase A: P table = nf @ W1s  (row-major bf16 in DRAM) ----
        NJ = NPAD // 128 if "A" in _DBG_PHASES else 0
        for j0 in range(0, NJ, 4):
            jn = min(4, NJ - j0)
            ps = psA.tile([128, 512], f32, tag="A")
            nfc = wk3.tile([128, 4, 128], bf, tag="nfc")
            for q in range(jn):
                j = j0 + q
                nc.sync.dma_start(nfc[:, q, :], nfT[:, j * 128:(j + 1) * 128])
                nc.tensor.matmul(out=ps[:, q * 128:(q + 1) * 128],
                                 lhsT=nfc[:, q, :], rhs=W1s_sb[:],
                                 start=True, stop=True)
            pc = wk3.tile([128, 4, 128], bf, tag="pc")
            nc.any.tensor_copy(pc[:, :jn, :].rearrange("p a b -> p (a b)"),
                               ps[:, : jn * 128])
            if not os.environ.get("K_NO_PTAB_DMA"):
                nc.sync.dma_start(
                    P_tab[j0 * 128:(j0 + jn) * 128, :].rearrange(
                        "(a p) d -> p a d", p=128),
                    pc[:, :jn, :])

        # ---- Phase B: Q blocks = nf_own @ W1d (node-major, resident) ----
        Q_sb = const.tile([128, NBLK, 128], bf)
        for b in range(NBLK if "B" in _DBG_PHASES else 0):
            ps = psA.tile([128, 128], f32, tag="A")
            nc.tensor.matmul(out=ps[:], lhsT=nfo_sb[:, b * 128:(b + 1) * 128],
                             rhs=W1d_sb[:], start=True, stop=True)
            nc.any.tensor_copy(Q_sb[:, b, :], ps[:])

        # ---- output staging ----
        outn_sb = const.tile([128, NBLK, D], f32)
        outc_sb = const.tile([128, NBLK, 3], f32)
        nc.vector.memset(outn_sb[:], 0.0)
        nc.vector.memset(outc_sb[:], 0.0)

        # ---- Phase C: per-block edge pipeline ----
        for b in range(min(NBLK, _DBG_NBLK) if "C" in _DBG_PHASES else 0):
            e0 = b * EPB
            idx_sl = src_idx_sb[:, b * (EPB // 16):(b + 1) * (EPB // 16)]

            Pg = work.tile([128, 1, EPB], bf, tag="Pg")
            nc.gpsimd.dma_gather(
                out_ap=Pg[:], in_ap=P_tab[:], idxs_ap=idx_sl,
                num_idxs=EPB, num_idxs_reg=EPB, elem_size=128, transpose=True,
                single_packet=False)
            Cg = work.tile([128, TPB, 64], f32, tag="Cg")
            nc.gpsimd.dma_gather(
                out_ap=Cg[:], in_ap=coords_tab[:], idxs_ap=idx_sl,
                num_idxs=EPB, num_idxs_reg=EPB, elem_size=64, transpose=False,
                single_packet=False)

            ea_sb = work.tile([EA, EPB], bf, tag="ea")
            nc.sync.dma_start(ea_sb[:], eaT[:, e0:e0 + EPB])
            drep = work.tile([128, EPB], bf, tag="drep")
            rep_ap = bass.AP(tensor=dstb_fm.tensor, offset=dstb_fm.offset + e0,
                             ap=[[0, 128], [1, EPB]])
            nc.gpsimd.dma_start(out=drep[:], in_=rep_ap)

            Mn_em = work.tile([128, TPB, 128], bf, tag="mnem")
            nc.any.tensor_tensor(
                out=Mn_em[:],
                in0=dem_sb[:, b * TPB:(b + 1) * TPB].unsqueeze(2).to_broadcast(
                    [128, TPB, 128]),
                in1=iota_rep[:].unsqueeze(1).to_broadcast([128, TPB, 128]),
                op=OP.is_equal)

            payload = work.tile([128, TPB, 132], bf, tag="pay")
            cd_sb = work.tile([128, TPB, 8], bf, tag="cdsb")

            for c in range(CPB):
                sl = slice(c * CH, (c + 1) * CH)
                mnfm = work.tile([128, CH], bf, tag="mnfm")
                nc.any.tensor_tensor(
                    out=mnfm[:],
                    in0=iota_col[:].to_broadcast([128, CH]),
                    in1=drep[:, sl], op=OP.is_equal)

                h1 = psA.tile([128, CH], f32, tag="A")
                nc.tensor.matmul(out=h1[:], lhsT=W1e_sb[:], rhs=ea_sb[:, sl],
                                 start=True, stop=False)
                nc.tensor.matmul(out=h1[:], lhsT=Q_sb[:, b, :], rhs=mnfm[:],
                                 start=False, stop=True)
                u = work.tile([128, CH], f32, tag="u")
                nc.any.tensor_tensor(out=u[:], in0=h1[:], in1=Pg[:, 0, sl],
                                     op=OP.add)
                h1s = work.tile([128, CH], bf, tag="h1s")
                nc.scalar.activation(h1s[:], u[:], AF.Silu, bias=eb1_sb[:])

                h2 = psB.tile([128, CH], f32, tag="B")
                nc.tensor.matmul(out=h2[:], lhsT=eW2_sb[:], rhs=h1s[:],
                                 start=True, stop=True)
                h2s = work.tile([128, CH], bf, tag="h2s")
                nc.scalar.activation(h2s[:], h2[:], AF.Silu, bias=eb2_sb[:])

                h3 = psS.tile([64, CH], f32, tag="S")
                nc.tensor.matmul(out=h3[:], lhsT=cW1_sb[:], rhs=h2s[:],
                                 start=True, stop=True)
                h3s = work.tile([64, CH], bf, tag="h3s")
                nc.scalar.activation(h3s[:], h3[:], AF.Silu, bias=cb1_sb[:])

                p8 = psS.tile([8, CH], f32, tag="S")
                nc.tensor.matmul(out=p8[:], lhsT=cW2p8_sb[:], rhs=h3s[:],
                                 start=True, stop=False)
                nc.tensor.matmul(out=p8[:], lhsT=chi8_sb[:, b, :], rhs=mnfm[:],
                                 start=False, stop=False)
                nc.tensor.matmul(out=p8[:], lhsT=clo8_sb[:, b, :], rhs=mnfm[:],
                                 start=False, stop=True)
                bund = work.tile([8, CH], bf, tag="bund")
                nc.any.tensor_copy(bund[:], p8[:])

                msgT = psT.tile([128, CH], bf, tag="T")
                cdT = psS.tile([128, 32], bf, tag="S")
                for t in range(4):
                    nc.tensor.transpose(out=msgT[:, t * 128:(t + 1) * 128],
                                        in_=h2s[:, t * 128:(t + 1) * 128],
                                        identity=ident_bf[:])
                    nc.tensor.transpose(out=cdT[:, t * 8:(t + 1) * 8],
                                        in_=bund[0:8, t * 128:(t + 1) * 128],
                                        identity=ident_bf[0:8, 0:8])
                nc.any.tensor_copy(
                    payload[:, c * 4:(c + 1) * 4, 0:128],
                    msgT[:].rearrange("p (t n) -> p t n", t=4))
                nc.any.tensor_copy(
                    cd_sb[:, c * 4:(c + 1) * 4, :],
                    cdT[:].rearrange("p (t n) -> p t n", t=4))

            # ---- block coord math (edge-major, f32) ----
            cdst = work.tile([128, TPB, 3], f32, tag="cdst")
            nc.any.tensor_tensor(out=cdst[:], in0=cd_sb[:, :, 1:4],
                                 in1=cd_sb[:, :, 4:7], op=OP.add)
            diff = work.tile([128, TPB, 3], f32, tag="diff")
            nc.any.tensor_tensor(out=diff[:], in0=Cg[:, :, 0:3], in1=cdst[:],
                                 op=OP.subtract)
            dsq = work.tile([128, TPB, 3], f32, tag="dsq")
            nc.any.tensor_tensor(out=dsq[:], in0=diff[:], in1=diff[:],
                                 op=OP.mult)
            ss = work.tile([128, TPB], f32, tag="ss")
            nc.vector.tensor_reduce(out=ss[:], in_=dsq[:], axis=AX.X, op=OP.add)
            nrm = work.tile([128, TPB], f32, tag="nrm")
            nc.scalar.activation(nrm[:], ss[:], AF.Sqrt)
            nrme = work.tile([128, TPB], f32, tag="nrme")
            nc.any.tensor_scalar_add(nrme[:], nrm[:], EPS)
            inv = work.tile([128, TPB], f32, tag="inv")
            nc.vector.reciprocal(inv[:], nrme[:])
            cwf = work.tile([128, TPB, 1], f32, tag="cwf")
            nc.any.tensor_scalar(out=cwf[:], in0=cd_sb[:, :, 0:1],
                                 scalar1=cb2_sb[:], scalar2=None, op0=OP.add)
            scal = work.tile([128, TPB, 1], f32, tag="scal")
            nc.any.tensor_tensor(out=scal[:], in0=inv[:].unsqueeze(2),
                                 in1=cwf[:], op=OP.mult)
            nc.any.tensor_tensor(out=payload[:, :, 128:131], in0=diff[:],
                                 in1=scal[:].to_broadcast([128, TPB, 3]),
                                 op=OP.mult)

            # ---- scatter: aggregate messages + coord updates per node ----
            sc = psC.tile([128, 131], f32, tag="C")
            for t in range(TPB):
                nc.tensor.matmul(out=sc[:], lhsT=Mn_em[:, t, :],
                                 rhs=payload[:, t, 0:131],
                                 start=(t == 0), stop=(t == TPB - 1))

            # ---- node update for this block ----
            aggr_bf = work.tile([128, 128], bf, tag="aggrbf")
            nc.any.tensor_copy(aggr_bf[:], sc[:, 0:128])
            agT_ps = psT.tile([128, 128], bf, tag="T")
            nc.tensor.transpose(out=agT_ps[:], in_=aggr_bf[:],
                                identity=ident_bf[:])
            agT = work.tile([128, 128], bf, tag="agT")
            nc.any.tensor_copy(agT[:], agT_ps[:])

            n1 = psA.tile([128, 128], f32, tag="A")
            nc.tensor.matmul(out=n1[:], lhsT=nW1a_sb[:],
                             rhs=nfo_sb[:, b * 128:(b + 1) * 128],
                             start=True, stop=False)
            nc.tensor.matmul(out=n1[:], lhsT=nW1b_sb[:], rhs=agT[:],
                             start=False, stop=True)
            n1s = work.tile([128, 128], bf, tag="n1s")
            nc.scalar.activation(n1s[:], n1[:], AF.Silu, bias=nb1_sb[:])
            n2 = psB.tile([128, 128], f32, tag="B")
            nc.tensor.matmul(out=n2[:], lhsT=nW2_sb[:], rhs=n1s[:],
                             start=True, stop=True)
            ups = work.tile([128, 128], f32, tag="ups")
            nc.scalar.activation(ups[:], n2[:], AF.Identity, bias=nb2_sb[:])
            updT = psT.tile([128, 128], f32, tag="T")
            nc.tensor.transpose(out=updT[:], in_=ups[:], identity=ident_f32[:])
            nc.any.tensor_tensor(out=outn_sb[:, b, :], in0=updT[:],
                                 in1=nfown_sb[:, b, :], op=OP.add)
            nc.any.tensor_tensor(out=outc_sb[:, b, :], in0=sc[:, 128:131],
                                 in1=coown_sb[:, b, :], op=OP.add)

        nc.sync.dma_start(
            out_nodes[:].rearrange("(a p) d -> p a d", p=128), outn_sb[:])
        nc.sync.dma_start(
            out_coords[:].rearrange("(a p) d -> p a d", p=128), outc_sb[:])

        pass

    nc.compile()
    return nc


def _get_program():
    if "nc" not in _PROG_CACHE:
        _PROG_CACHE["nc"] = _build_program()
    return _PROG_CACHE["nc"]


def kernel(**inputs):
    in_maps = _prep(**inputs)
    nc = _get_program()
    res = run_bass_kernel_spmd(nc, in_maps, core_ids=list(range(NCORES)))
    nodes = np.concatenate([res.results[c]["out_nodes"] for c in range(NCORES)],
                           axis=0)[:NN]
    coords = np.concatenate([res.results[c]["out_coords"] for c in range(NCORES)],
                            axis=0)[:NN]
    return nodes, coords


# revision 23
# speedup vs baseline: 41.0487x; 41.0487x over previous
"""EquivariantGraphConv on 8 Trainium2 NeuronCores.

Strategy (edge-parallel, dst-ownership sharding):
  - Nodes are partitioned into 8 contiguous ranges of 2560 (core 7 padded).
  - Each core owns 20 node blocks of 128; every edge is routed to the core
    (and 128-node block) that owns its dst, so segment sums need no
    cross-core reduction at all.
  - Host pads each (core, block) edge list to a fixed capacity so the
    compiled program is identical on all cores (SPMD).
  - On device, per edge block: gather src features via dma_gather from a
    precomputed table P = node_feat @ eW1[:128] (built on device), run the
    edge MLP feature-major in bf16, build one-hot dst matrices with
    iota/is_equal and use PE matmuls for the dst gathers (Q = nf @ eW1[128:256],
    coords[dst]) and for the segment-sum scatter (PSUM accumulation).
  - Node MLP + residuals run per block; outputs are returned as per-core
    shards and concatenated on the host.
"""

import sys

sys.path.insert(0, "/opt/trn_rl_repo")

import numpy as np
import ml_dtypes

import concourse.bass as bass
import concourse.bacc as bacc
import concourse.mybir as mybir
import concourse.tile as tile
from concourse.bass_utils import run_bass_kernel_spmd

BF16 = ml_dtypes.bfloat16
F32 = np.float32

# Problem shapes (hardcoded per spec)
NN = 20000          # nodes
E = 640000          # edges
D = 128             # node feat dim == HID
EA = 64             # edge attr dim
HID = 128
EPS = 1e-8

NCORES = 8
NBLK = 20           # node blocks per core
NPC = NBLK * 128    # nodes per core (2560)
GBLK = 160          # padded global block count (157 real)
NPAD = 157 * 128    # padded node count for tables (20096)
EPB = 4608          # edge capacity per (core, block); Poisson(4096)+8sigma
TPB = EPB // 128    # 36 tiles / block
CPB = EPB // 512    # 9 chunks / block
CH = 512
EPC = NBLK * EPB    # 92160 edges per core

_PROG_CACHE = {}

import os
_DBG_PHASES = os.environ.get("K_PHASES", "ABC")  # subset of "ABC" for bisection
_DBG_NBLK = int(os.environ.get("K_NBLK", str(NBLK)))
_DBG_CLVL = int(os.environ.get("K_CLVL", "7"))


def _to_gather_idx_layout(idx_i16: np.ndarray) -> np.ndarray:
    """dma_gather index layout: logical i -> partition i%16, col i//16,
    replicated across the 8 groups of 16 partitions."""
    s = idx_i16.reshape(-1, 16).T  # [16, n/16]
    return np.ascontiguousarray(np.tile(s, (8, 1)))  # [128, n/16]


def _prep(node_feat, edge_index, edge_attr, coords,
          eW1, eb1, eW2, eb2, nW1, nb1, nW2, nb2, cW1, cb1, cW2, cb2):
    node_feat = np.asarray(node_feat, dtype=F32)
    edge_index = np.asarray(edge_index)
    edge_attr = np.asarray(edge_attr, dtype=F32)
    coords = np.asarray(coords, dtype=F32)

    src = np.asarray(edge_index[0], dtype=np.int64)
    dst = np.asarray(edge_index[1], dtype=np.int64)

    g = dst >> 7                      # global 128-node block, 0..156
    order = np.argsort(g, kind="stable")
    counts = np.bincount(g, minlength=GBLK)
    if counts.max() > EPB:
        raise RuntimeError(f"block overflow: {counts.max()} > {EPB}")
    off = np.zeros(GBLK + 1, dtype=np.int64)
    off[1:] = np.cumsum(counts)

    es = order
    gs = g[es]
    pos = np.arange(E, dtype=np.int64) - off[gs]
    slot = gs * EPB + pos             # slot in [0, GBLK*EPB)

    TOT = GBLK * EPB
    src_pad = np.zeros(TOT, dtype=np.int16)
    dstb_pad = np.full(TOT, 192.0, dtype=F32)
    ea_pad = np.zeros((TOT, EA), dtype=BF16)
    src_pad[slot] = src[es].astype(np.int16)
    dstb_pad[slot] = (dst[es] & 127).astype(F32)
    ea_pad[slot] = edge_attr[es].astype(BF16)

    # node tables (shared across cores)
    nfT = np.zeros((D, NPAD), dtype=BF16)
    nfT[:, :NN] = node_feat.T.astype(BF16)
    coords_tab = np.zeros((NPAD, 64), dtype=F32)
    coords_tab[:NN, :3] = coords

    iota_col = np.arange(128, dtype=F32).reshape(128, 1).astype(BF16)
    iota_rep = np.broadcast_to(np.arange(128, dtype=F32).astype(BF16), (128, 128)).copy()
    ident_bf = np.eye(128, dtype=F32).astype(BF16)
    ident_f32 = np.eye(128, dtype=F32)

    # weights
    W1s = np.ascontiguousarray(eW1[0:128]).astype(BF16)         # [128,128]
    W1d = np.ascontiguousarray(eW1[128:256]).astype(BF16)       # [128,128]
    W1e = np.ascontiguousarray(eW1[256:320]).astype(BF16)       # [64,128]
    eW2b = np.asarray(eW2, dtype=F32).astype(BF16)
    cW1b = np.asarray(cW1, dtype=F32).astype(BF16)              # [128,64]
    cW2p8 = np.zeros((64, 8), dtype=BF16)
    cW2p8[:, 0] = np.asarray(cW2, dtype=F32)[:, 0].astype(BF16)
    nW1a = np.ascontiguousarray(np.asarray(nW1, dtype=F32)[0:128]).astype(BF16)
    nW1b = np.ascontiguousarray(np.asarray(nW1, dtype=F32)[128:256]).astype(BF16)
    nW2b = np.asarray(nW2, dtype=F32).astype(BF16)

    shared = {
        "nfT": nfT,
        "coords_tab": coords_tab,
        "iota_col": iota_col,
        "iota_rep": iota_rep,
        "ident_bf": ident_bf,
        "ident_f32": ident_f32,
        "W1s": W1s, "W1d": W1d, "W1e": W1e, "eW2": eW2b,
        "cW1": cW1b, "cW2p8": cW2p8,
        "nW1a": nW1a, "nW1b": nW1b, "nW2": nW2b,
        "eb1": np.asarray(eb1, F32).reshape(HID, 1),
        "eb2": np.asarray(eb2, F32).reshape(HID, 1),
        "cb1": np.asarray(cb1, F32).reshape(64, 1),
        "cb2": np.full((128, 1), np.asarray(cb2, F32).reshape(-1)[0], dtype=F32),
        "nb1": np.asarray(nb1, F32).reshape(HID, 1),
        "nb2": np.asarray(nb2, F32).reshape(D, 1),
    }

    in_maps = []
    for c in range(NCORES):
        lo = c * NBLK * EPB
        hi = lo + NBLK * EPB
        src_c = src_pad[lo:hi]
        dstb_c = dstb_pad[lo:hi]
        ea_c = ea_pad[lo:hi]

        # per-core owned-node slices (core 7 zero-padded)
        n0 = c * NPC
        n1 = min((c + 1) * NPC, NN)
        nfT_own = np.zeros((D, NPC), dtype=BF16)
        nfT_own[:, : n1 - n0] = node_feat[n0:n1].T.astype(BF16)
        nf_own = np.zeros((NPC, D), dtype=F32)
        nf_own[: n1 - n0] = node_feat[n0:n1]
        co_own = np.zeros((NPC, 3), dtype=F32)
        co_own[: n1 - n0] = coords[n0:n1]

        chi = co_own.astype(BF16)
        clo = (co_own - chi.astype(F32)).astype(BF16)
        chi8 = np.zeros((NPC, 8), dtype=BF16)
        clo8 = np.zeros((NPC, 8), dtype=BF16)
        chi8[:, 1:4] = chi
        clo8[:, 4:7] = clo

        m = {
            "src_idx": _to_gather_idx_layout(src_c),
            "dstb_fm": np.ascontiguousarray(dstb_c.astype(BF16).reshape(1, EPC)),
            "dstb_em": np.ascontiguousarray(dstb_c.astype(BF16).reshape(-1, 128).T),
            "eaT": np.ascontiguousarray(ea_c.T),
            "nfT_own": nfT_own,
            "nf_own_t": np.ascontiguousarray(
                nf_own.reshape(NBLK, 128, D).transpose(1, 0, 2)),
            "co_own_t": np.ascontiguousarray(
                co_own.reshape(NBLK, 128, 3).transpose(1, 0, 2)),
            "chi8_t": np.ascontiguousarray(
                chi8.reshape(NBLK, 128, 8).transpose(1, 0, 2)),
            "clo8_t": np.ascontiguousarray(
                clo8.reshape(NBLK, 128, 8).transpose(1, 0, 2)),
        }
        m.update(shared)
        in_maps.append(m)
    return in_maps


def _build_program():
    nc = bacc.Bacc("TRN2", target_bir_lowering=False, debug=False,
                   num_devices=NCORES)
    bf = mybir.dt.bfloat16
    f32 = mybir.dt.float32
    i16 = mybir.dt.int16

    def inp(name, shape, dt):
        return nc.dram_tensor(name, list(shape), dt, kind="ExternalInput").ap()

    src_idx = inp("src_idx", (128, EPC // 16), i16)
    dstb_fm = inp("dstb_fm", (1, EPC), bf)
    dstb_em = inp("dstb_em", (128, EPC // 128), bf)
    eaT = inp("eaT", (EA, EPC), bf)
    nfT = inp("nfT", (D, NPAD), bf)
    nfT_own = inp("nfT_own", (D, NPC), bf)
    nf_own_t = inp("nf_own_t", (128, NBLK, D), f32)
    co_own_t = inp("co_own_t", (128, NBLK, 3), f32)
    chi8_t = inp("chi8_t", (128, NBLK, 8), bf)
    clo8_t = inp("clo8_t", (128, NBLK, 8), bf)
    coords_tab = inp("coords_tab", (NPAD, 64), f32)
    iota_col_i = inp("iota_col", (128, 1), bf)
    iota_rep_i = inp("iota_rep", (128, 128), bf)
    ident_bf_i = inp("ident_bf", (128, 128), bf)
    ident_f32_i = inp("ident_f32", (128, 128), f32)
    W1s_i = inp("W1s", (128, 128), bf)
    W1d_i = inp("W1d", (128, 128), bf)
    W1e_i = inp("W1e", (64, 128), bf)
    eW2_i = inp("eW2", (128, 128), bf)
    cW1_i = inp("cW1", (128, 64), bf)
    cW2p8_i = inp("cW2p8", (64, 8), bf)
    nW1a_i = inp("nW1a", (128, 128), bf)
    nW1b_i = inp("nW1b", (128, 128), bf)
    nW2_i = inp("nW2", (128, 128), bf)
    eb1_i = inp("eb1", (HID, 1), f32)
    eb2_i = inp("eb2", (HID, 1), f32)
    cb1_i = inp("cb1", (64, 1), f32)
    cb2_i = inp("cb2", (128, 1), f32)
    nb1_i = inp("nb1", (HID, 1), f32)
    nb2_i = inp("nb2", (D, 1), f32)

    out_nodes = nc.dram_tensor("out_nodes", [NPC, D], f32,
                               kind="ExternalOutput").ap()
    out_coords = nc.dram_tensor("out_coords", [NPC, 3], f32,
                                kind="ExternalOutput").ap()
    P_tab = nc.dram_tensor("P_tab", [NPAD, D], bf, kind="Internal").ap()

    AX = mybir.AxisListType
    OP = mybir.AluOpType
    AF = mybir.ActivationFunctionType

    from contextlib import ExitStack
    _trace_sim = bool(os.environ.get("K_TRACE_SIM"))
    with tile.TileContext(nc, trace_sim=_trace_sim) as tc, ExitStack() as stk:
        const = stk.enter_context(tc.tile_pool(name="const", bufs=1))
        work = stk.enter_context(tc.tile_pool(name="work", bufs=2))
        wk3 = stk.enter_context(tc.tile_pool(name="wk3", bufs=3))
        psA = stk.enter_context(tc.tile_pool(name="psA", bufs=2, space="PSUM"))
        psB = stk.enter_context(tc.tile_pool(name="psB", bufs=2, space="PSUM"))
        psS = stk.enter_context(tc.tile_pool(name="psS", bufs=2, space="PSUM"))
        psT = stk.enter_context(tc.tile_pool(name="psT", bufs=1, space="PSUM"))
        psC = stk.enter_context(tc.tile_pool(name="psC", bufs=1, space="PSUM"))

        def cload(ap_in, shape, dtype):
            t = const.tile(shape, dtype, tag=f"c_{ap_in.tensor.name}")
            nc.sync.dma_start(t[:], ap_in[:])
            return t

        W1s_sb = cload(W1s_i, [128, 128], bf)
        W1d_sb = cload(W1d_i, [128, 128], bf)
        W1e_sb = cload(W1e_i, [64, 128], bf)
        eW2_sb = cload(eW2_i, [128, 128], bf)
        cW1_sb = cload(cW1_i, [128, 64], bf)
        cW2p8_sb = cload(cW2p8_i, [64, 8], bf)
        nW1a_sb = cload(nW1a_i, [128, 128], bf)
        nW1b_sb = cload(nW1b_i, [128, 128], bf)
        nW2_sb = cload(nW2_i, [128, 128], bf)
        eb1_sb = cload(eb1_i, [HID, 1], f32)
        eb2_sb = cload(eb2_i, [HID, 1], f32)
        cb1_sb = cload(cb1_i, [64, 1], f32)
        cb2_sb = cload(cb2_i, [128, 1], f32)
        nb1_sb = cload(nb1_i, [HID, 1], f32)
        nb2_sb = cload(nb2_i, [D, 1], f32)
        iota_col = cload(iota_col_i, [128, 1], bf)
        iota_rep = cload(iota_rep_i, [128, 128], bf)
        ident_bf = cload(ident_bf_i, [128, 128], bf)
        ident_f32 = cload(ident_f32_i, [128, 128], f32)
        src_idx_sb = cload(src_idx, [128, EPC // 16], i16)
        nfo_sb = cload(nfT_own, [D, NPC], bf)
        nfown_sb = cload(nf_own_t, [128, NBLK, D], f32)
        coown_sb = cload(co_own_t, [128, NBLK, 3], f32)
        chi8_sb = cload(chi8_t, [128, NBLK, 8], bf)
        clo8_sb = cload(clo8_t, [128, NBLK, 8], bf)
        dem_sb = cload(dstb_em, [128, EPC // 128], bf)

        # ---- Phase A: P table = nf @ W1s  (row-major bf16 in DRAM) ----
        NJ = NPAD // 128 if "A" in _DBG_PHASES else 0
        for j0 in range(0, NJ, 4):
            jn = min(4, NJ - j0)
            ps = psA.tile([128, 512], f32, tag="A")
            nfc = wk3.tile([128, 4, 128], bf, tag="nfc")
            for q in range(jn):
                j = j0 + q
                nc.sync.dma_start(nfc[:, q, :], nfT[:, j * 128:(j + 1) * 128])
                nc.tensor.matmul(out=ps[:, q * 128:(q + 1) * 128],
                                 lhsT=nfc[:, q, :], rhs=W1s_sb[:],
                                 start=True, stop=True)
            pc = wk3.tile([128, 4, 128], bf, tag="pc")
            nc.vector.tensor_copy(pc[:, :jn, :].rearrange("p a b -> p (a b)"),
                               ps[:, : jn * 128])
            if not os.environ.get("K_NO_PTAB_DMA"):
                nc.sync.dma_start(
                    P_tab[j0 * 128:(j0 + jn) * 128, :].rearrange(
                        "(a p) d -> p a d", p=128),
                    pc[:, :jn, :])

        # ---- Phase B: Q blocks = nf_own @ W1d (node-major, resident) ----
        Q_sb = const.tile([128, NBLK, 128], bf)
        for b in range(NBLK if "B" in _DBG_PHASES else 0):
            ps = psA.tile([128, 128], f32, tag="A")
            nc.tensor.matmul(out=ps[:], lhsT=nfo_sb[:, b * 128:(b + 1) * 128],
                             rhs=W1d_sb[:], start=True, stop=True)
            nc.vector.tensor_copy(Q_sb[:, b, :], ps[:])

        # ---- output staging ----
        outn_sb = const.tile([128, NBLK, D], f32)
        outc_sb = const.tile([128, NBLK, 3], f32)
        nc.vector.memset(outn_sb[:], 0.0)
        nc.vector.memset(outc_sb[:], 0.0)

        # ---- Phase C: per-block edge pipeline ----
        for b in range(min(NBLK, _DBG_NBLK) if "C" in _DBG_PHASES else 0):
            e0 = b * EPB
            idx_sl = src_idx_sb[:, b * (EPB // 16):(b + 1) * (EPB // 16)]

            Pg = work.tile([128, 1, EPB], bf, tag="Pg")
            nc.gpsimd.dma_gather(
                out_ap=Pg[:], in_ap=P_tab[:], idxs_ap=idx_sl,
                num_idxs=EPB, num_idxs_reg=EPB, elem_size=128, transpose=True,
                single_packet=False)
            Cg = work.tile([128, TPB, 64], f32, tag="Cg")
            nc.gpsimd.dma_gather(
                out_ap=Cg[:], in_ap=coords_tab[:], idxs_ap=idx_sl,
                num_idxs=EPB, num_idxs_reg=EPB, elem_size=64, transpose=False,
                single_packet=False)

            ea_sb = work.tile([EA, EPB], bf, tag="ea")
            nc.sync.dma_start(ea_sb[:], eaT[:, e0:e0 + EPB])
            drep = work.tile([128, EPB], bf, tag="drep")
            rep_ap = bass.AP(tensor=dstb_fm.tensor, offset=dstb_fm.offset + e0,
                             ap=[[0, 128], [1, EPB]])
            nc.gpsimd.dma_start(out=drep[:], in_=rep_ap)

            Mn_em = work.tile([128, TPB, 128], bf, tag="mnem")
            nc.any.tensor_tensor(
                out=Mn_em[:],
                in0=dem_sb[:, b * TPB:(b + 1) * TPB].unsqueeze(2).to_broadcast(
                    [128, TPB, 128]),
                in1=iota_rep[:].unsqueeze(1).to_broadcast([128, TPB, 128]),
                op=OP.is_equal)

            payload = work.tile([128, TPB, 132], bf, tag="pay")
            cd_sb = work.tile([128, TPB, 8], bf, tag="cdsb")

            for c in range(CPB):
                sl = slice(c * CH, (c + 1) * CH)
                mnfm = work.tile([128, CH], bf, tag="mnfm")
                nc.any.tensor_tensor(
                    out=mnfm[:],
                    in0=iota_col[:].to_broadcast([128, CH]),
                    in1=drep[:, sl], op=OP.is_equal)

                h1 = psA.tile([128, CH], f32, tag="A")
                nc.tensor.matmul(out=h1[:], lhsT=W1e_sb[:], rhs=ea_sb[:, sl],
                                 start=True, stop=False)
                nc.tensor.matmul(out=h1[:], lhsT=Q_sb[:, b, :], rhs=mnfm[:],
                                 start=False, stop=True)
                u = work.tile([128, CH], f32, tag="u")
                nc.any.tensor_tensor(out=u[:], in0=h1[:], in1=Pg[:, 0, sl],
                                     op=OP.add)
                h1s = work.tile([128, CH], bf, tag="h1s")
                nc.scalar.activation(h1s[:], u[:], AF.Silu, bias=eb1_sb[:])

                h2 = psB.tile([128, CH], f32, tag="B")
                nc.tensor.matmul(out=h2[:], lhsT=eW2_sb[:], rhs=h1s[:],
                                 start=True, stop=True)
                h2s = work.tile([128, CH], bf, tag="h2s")
                nc.scalar.activation(h2s[:], h2[:], AF.Silu, bias=eb2_sb[:])

                h3 = psS.tile([64, CH], f32, tag="S")
                nc.tensor.matmul(out=h3[:], lhsT=cW1_sb[:], rhs=h2s[:],
                                 start=True, stop=True)
                h3s = work.tile([64, CH], bf, tag="h3s")
                nc.scalar.activation(h3s[:], h3[:], AF.Silu, bias=cb1_sb[:])

                p8 = psS.tile([8, CH], f32, tag="S")
                nc.tensor.matmul(out=p8[:], lhsT=cW2p8_sb[:], rhs=h3s[:],
                                 start=True, stop=False)
                nc.tensor.matmul(out=p8[:], lhsT=chi8_sb[:, b, :], rhs=mnfm[:],
                                 start=False, stop=False)
                nc.tensor.matmul(out=p8[:], lhsT=clo8_sb[:, b, :], rhs=mnfm[:],
                                 start=False, stop=True)
                bund = work.tile([8, CH], bf, tag="bund")
                nc.vector.tensor_copy(bund[:], p8[:])

                msgT = psT.tile([128, CH], bf, tag="T")
                cdT = psS.tile([128, 32], bf, tag="S")
                for t in range(4):
                    nc.tensor.transpose(out=msgT[:, t * 128:(t + 1) * 128],
                                        in_=h2s[:, t * 128:(t + 1) * 128],
                                        identity=ident_bf[:])
                    nc.tensor.transpose(out=cdT[:, t * 8:(t + 1) * 8],
                                        in_=bund[0:8, t * 128:(t + 1) * 128],
                                        identity=ident_bf[0:8, 0:8])
                nc.vector.tensor_copy(
                    payload[:, c * 4:(c + 1) * 4, 0:128],
                    msgT[:].rearrange("p (t n) -> p t n", t=4))
                nc.vector.tensor_copy(
                    cd_sb[:, c * 4:(c + 1) * 4, :],
                    cdT[:].rearrange("p (t n) -> p t n", t=4))

            # ---- block coord math (edge-major, f32) ----
            cdst = work.tile([128, TPB, 3], f32, tag="cdst")
            nc.any.tensor_tensor(out=cdst[:], in0=cd_sb[:, :, 1:4],
                                 in1=cd_sb[:, :, 4:7], op=OP.add)
            diff = work.tile([128, TPB, 3], f32, tag="diff")
            nc.any.tensor_tensor(out=diff[:], in0=Cg[:, :, 0:3], in1=cdst[:],
                                 op=OP.subtract)
            dsq = work.tile([128, TPB, 3], f32, tag="dsq")
            nc.any.tensor_tensor(out=dsq[:], in0=diff[:], in1=diff[:],
                                 op=OP.mult)
            ss = work.tile([128, TPB], f32, tag="ss")
            nc.vector.tensor_reduce(out=ss[:], in_=dsq[:], axis=AX.X, op=OP.add)
            nrm = work.tile([128, TPB], f32, tag="nrm")
            nc.scalar.activation(nrm[:], ss[:], AF.Sqrt)
            nrme = work.tile([128, TPB], f32, tag="nrme")
            nc.any.tensor_scalar_add(nrme[:], nrm[:], EPS)
            inv = work.tile([128, TPB], f32, tag="inv")
            nc.vector.reciprocal(inv[:], nrme[:])
            cwf = work.tile([128, TPB, 1], f32, tag="cwf")
            nc.any.tensor_scalar(out=cwf[:], in0=cd_sb[:, :, 0:1],
                                 scalar1=cb2_sb[:], scalar2=None, op0=OP.add)
            scal = work.tile([128, TPB, 1], f32, tag="scal")
            nc.any.tensor_tensor(out=scal[:], in0=inv[:].unsqueeze(2),
                                 in1=cwf[:], op=OP.mult)
            nc.any.tensor_tensor(out=payload[:, :, 128:131], in0=diff[:],
                                 in1=scal[:].to_broadcast([128, TPB, 3]),
                                 op=OP.mult)

            # ---- scatter: aggregate messages + coord updates per node ----
            sc = psC.tile([128, 131], f32, tag="C")
            for t in range(TPB):
                nc.tensor.matmul(out=sc[:], lhsT=Mn_em[:, t, :],
                                 rhs=payload[:, t, 0:131],
                                 start=(t == 0), stop=(t == TPB - 1))

            # ---- node update for this block ----
            aggr_bf = work.tile([128, 128], bf, tag="aggrbf")
            nc.vector.tensor_copy(aggr_bf[:], sc[:, 0:128])
            agT_ps = psT.tile([128, 128], bf, tag="T")
            nc.tensor.transpose(out=agT_ps[:], in_=aggr_bf[:],
                                identity=ident_bf[:])
            agT = work.tile([128, 128], bf, tag="agT")
            nc.vector.tensor_copy(agT[:], agT_ps[:])

            n1 = psA.tile([128, 128], f32, tag="A")
            nc.tensor.matmul(out=n1[:], lhsT=nW1a_sb[:],
                             rhs=nfo_sb[:, b * 128:(b + 1) * 128],
                             start=True, stop=False)
            nc.tensor.matmul(out=n1[:], lhsT=nW1b_sb[:], rhs=agT[:],
                             start=False, stop=True)
            n1s = work.tile([128, 128], bf, tag="n1s")
            nc.scalar.activation(n1s[:], n1[:], AF.Silu, bias=nb1_sb[:])
            n2 = psB.tile([128, 128], f32, tag="B")
            nc.tensor.matmul(out=n2[:], lhsT=nW2_sb[:], rhs=n1s[:],
                             start=True, stop=True)
            ups = work.tile([128, 128], f32, tag="ups")
            nc.scalar.activation(ups[:], n2[:], AF.Identity, bias=nb2_sb[:])
            updT = psT.tile([128, 128], f32, tag="T")
            nc.tensor.transpose(out=updT[:], in_=ups[:], identity=ident_f32[:])
            nc.any.tensor_tensor(out=outn_sb[:, b, :], in0=updT[:],
                                 in1=nfown_sb[:, b, :], op=OP.add)
            nc.any.tensor_tensor(out=outc_sb[:, b, :], in0=sc[:, 128:131],
                                 in1=coown_sb[:, b, :], op=OP.add)

        nc.sync.dma_start(
            out_nodes[:].rearrange("(a p) d -> p a d", p=128), outn_sb[:])
        nc.sync.dma_start(
            out_coords[:].rearrange("(a p) d -> p a d", p=128), outc_sb[:])

        pass

    nc.compile()
    return nc


def _get_program():
    if "nc" not in _PROG_CACHE:
        _PROG_CACHE["nc"] = _build_program()
    return _PROG_CACHE["nc"]


def _get_runner():
    """Build (once) a cached jitted shard_map callable over 8 cores.

    Mirrors concourse.bass2jax.run_bass_via_pjrt but keeps the jit alive so
    repeated kernel() calls skip retracing and allow pure-exec timing.
    """
    if "runner" in _PROG_CACHE:
        return _PROG_CACHE["runner"]
    nc = _get_program()
    import jax
    from jax.sharding import Mesh, PartitionSpec
    from jax.experimental.shard_map import shard_map
    from concourse import bass2jax, mybir as mb

    bass2jax.install_neuronx_cc_hook()

    partition_name = (nc.partition_id_tensor.name
                      if nc.partition_id_tensor else None)
    in_names, out_names, out_avals, zero_outs = [], [], [], []
    for alloc in nc.m.functions[0].allocations:
        if not isinstance(alloc, mb.MemoryLocationSet):
            continue
        name = alloc.memorylocations[0].name
        if alloc.kind == "ExternalInput":
            if name != partition_name:
                in_names.append(name)
        elif alloc.kind == "ExternalOutput":
            shape = tuple(alloc.tensor_shape)
            dtype = mb.dt.np(alloc.dtype)
            out_names.append(name)
            out_avals.append(jax.core.ShapedArray(shape, dtype))
            zero_outs.append(np.zeros(shape, dtype))
    n_params = len(in_names)
    all_names = in_names + out_names
    if partition_name is not None:
        all_names = all_names + [partition_name]

    def _body(*args):
        operands = list(args)
        if partition_name is not None:
            operands.append(bass2jax.partition_id_tensor())
        outs = bass2jax._bass_exec_p.bind(
            *operands,
            out_avals=tuple(out_avals),
            in_names=tuple(all_names),
            out_names=tuple(out_names),
            lowering_input_output_aliases=(),
            sim_require_finite=True,
            sim_require_nnan=True,
            nc=nc,
        )
        return tuple(outs)

    devices = jax.devices()[:NCORES]
    mesh = Mesh(np.asarray(devices), ("core",))
    n_outs = len(out_names)
    sharded = jax.jit(
        shard_map(_body, mesh=mesh,
                  in_specs=(PartitionSpec("core"),) * (n_params + n_outs),
                  out_specs=(PartitionSpec("core"),) * n_outs,
                  check_rep=False),
        keep_unused=True,
    )
    runner = {
        "fn": sharded,
        "in_names": in_names,
        "out_names": out_names,
        "zero_outs": zero_outs,
        "mesh": mesh,
    }
    _PROG_CACHE["runner"] = runner
    return runner


def _concat_inputs(runner, in_maps):
    ins = [np.concatenate([np.asarray(in_maps[c][n]) for c in range(NCORES)],
                          axis=0) for n in runner["in_names"]]
    zs = [np.zeros((NCORES * z.shape[0], *z.shape[1:]), z.dtype)
          for z in runner["zero_outs"]]
    return ins, zs


def _run(in_maps):
    runner = _get_runner()
    ins, zs = _concat_inputs(runner, in_maps)
    outs = runner["fn"](*ins, *zs)
    res = {}
    for i, n in enumerate(runner["out_names"]):
        arr = np.asarray(outs[i])
        res[n] = arr.reshape(NCORES, arr.shape[0] // NCORES, *arr.shape[1:])
    return res


def kernel(**inputs):
    in_maps = _prep(**inputs)
    res = _run(in_maps)
    nodes = res["out_nodes"].reshape(NCORES * NPC, D)[:NN]
    coords = res["out_coords"].reshape(NCORES * NPC, 3)[:NN]
    return nodes, coords


def bench_exec(inputs, iters=10):
    """Time pure device executions with inputs pre-staged on device."""
    import time
    import jax
    in_maps = _prep(**inputs)
    runner = _get_runner()
    ins, zs = _concat_inputs(runner, in_maps)
    fn = runner["fn"]
    outs = fn(*ins, *zs)   # warm-up (compile)
    jax.block_until_ready(outs)
    # pre-stage inputs on device
    from jax.sharding import NamedSharding, PartitionSpec
    sh = NamedSharding(runner["mesh"], PartitionSpec("core"))
    d_ins = [jax.device_put(x, sh) for x in ins]
    d_zs = [jax.device_put(z, sh) for z in zs]
    jax.block_until_ready(d_ins)
    ts = []
    for _ in range(iters):
        t0 = time.perf_counter()
        outs = fn(*d_ins, *d_zs)
        jax.block_until_ready(outs)
        ts.append(time.perf_counter() - t0)
    return ts
